# revision 32
# baseline (speedup 1.0000x reference)
"""Trainium2 Bass kernel for CombinedVectorField (CFG vector field + exact
Jacobian-trace divergence).

Math: with u = tanh(x@W1x + h@W1h + b1'), b1' = b1 + t*W1[256],
  v(x,h)  = u @ W2 + b2
  div(x,h)= sum_k (1-u_k^2) c_k = d0 - (u*u) @ c,   c_k = sum_i W1x[i,k] W2[k,i]
Output = concat[(1-gs)*v_null + gs*v_h, (1-gs)*div_null + gs*div_h].

Sharding: pure data parallel — each of the 8 cores takes 512 batch rows
(both guidance branches), weights replicated. All tensors are kept
feature-major (transposed) on device so every matmul contracts over the
partition dim; host does the transposes/reshapes only.
"""
import sys

sys.path.insert(0, "/opt/trn_rl_repo")

import ml_dtypes
import numpy as np

import concourse.bass as bass
import concourse.tile as tile
from concourse import bacc, mybir
from concourse.bass_utils import run_bass_kernel_spmd
from concourse.vector_clock import ScopedClock


class _TrimTileContext(tile.TileContext):
    """TileContext with the final all-engine barrier dropped from the
    teardown and the mid barrier reduced to sem-only (no per-engine
    drains). The head drain still waits for every semaphore (incl.
    output-DMA completion) and semaphores are still cleared for the next
    execution; only the trailing barrier (nothing executes after it) is
    elided."""

    def _drain_and_barrier(self, tick_clock, wait_clock):
        drain_inst = self.nc.sync.drain()
        wait_clock.add_sem_waits(
            drain_inst.ins, ScopedClock({None: tick_clock.global_clock})
        )
        self.nc.all_engine_barrier(sem_only=True)
        popped = self.nc._tile_sem_poison_stack.pop()
        assert popped is self._sem_poison
        self.nc.clear_and_free_semaphores(list(self.sems.allocated().values()))


class _FastBacc(bacc.Bacc):
    """Bacc whose constructor-time all-engine barrier (after the const-tile
    memsets) is sem-only — the per-engine drains there cost ~1us of kernel
    head time and order nothing we rely on beyond the memsets, which the
    event-semaphore barrier already orders."""

    def all_engine_barrier(self, *, sem_only: bool = False):
        super().all_engine_barrier(sem_only=True)

F32 = mybir.dt.float32
BF16 = mybir.dt.bfloat16
AF = mybir.ActivationFunctionType
ALU = mybir.AluOpType

N_CORES = 8
B = 4096
DIM_X = 128
DIM_H = 128
HIDDEN = 512
R = B // N_CORES          # rows per core
NCH = HIDDEN // 128       # hidden chunks
W2W = NCH * DIM_X + NCH   # w2 chunks + cmat columns

_NC_CACHE = None


def _build():
    nc = _FastBacc("TRN2", target_bir_lowering=False, debug=False,
                   enable_asserts=False, monotonic_sem_count=0)

    # four bf16 input blobs, alternating over the two HWDGE rings so the
    # first-matmul gate (A1 + B1) is as small as possible; the w2 blob (B2)
    # is only needed once the first tanh completes.
    #   A1 = [xT | w1x]   A2 = [w1h]   B1 = [hT | hnT]
    #   B2 = [gs*w2r | (1-gs)*w2r | -gs*cmat | -(1-gs)*cmat]
    # (guidance-scale combine folded into the weights on the host, so PSUM
    #  accumulates the already-combined v and div directly)
    inA1 = nc.dram_tensor("inA1", [128, R + HIDDEN], BF16, kind="ExternalInput")
    inA2 = nc.dram_tensor("inA2", [128, HIDDEN], BF16, kind="ExternalInput")
    inB1 = nc.dram_tensor("inB1", [128, 2 * R], BF16, kind="ExternalInput")
    inB2 = nc.dram_tensor("inB2", [128, 2 * W2W], BF16, kind="ExternalInput")
    # aux cols: 0-3 b1' chunks, 4 b2, 5 d0
    aux = nc.dram_tensor("aux", [128, 6], F32, kind="ExternalInput")

    VO = nc.dram_tensor("VO", [DIM_X, R], F32, kind="ExternalOutput")
    DO = nc.dram_tensor("DO", [1, R], F32, kind="ExternalOutput")

    with _TrimTileContext(nc) as tc:
        with tc.tile_pool(name="cst", bufs=1) as cst, \
             tc.tile_pool(name="act", bufs=3) as actp, \
             tc.tile_pool(name="out", bufs=1) as outp, \
             tc.tile_pool(name="psa", bufs=6, space="PSUM") as psa, \
             tc.tile_pool(name="psv", bufs=1, space="PSUM") as psv:
            # PE prewarm: dummy f32 matmuls on a zeroed tile keep the PE-HAM
            # activity window busy during the input DMAs, so real matmuls run
            # at 2.4 GHz instead of 1.2 GHz.
            wrm = cst.tile([128, 256], F32)
            nc.gpsimd.memset(wrm[:], 0.0)
            pwarm = psa.tile([128, R], F32, tag="a")
            for _ in range(5):
                nc.tensor.matmul(pwarm[:, 0:256], wrm[:, 0:128], wrm[:],
                                 start=True, stop=True, skip_group_check=True)

            # scalar ring issues first (sync's first DMA waits on a drain),
            # so the first-matmul gate (A1) goes there
            a1t = cst.tile([128, R + HIDDEN], BF16)
            nc.scalar.dma_start(out=a1t[:], in_=inA1[:])
            a2t = cst.tile([128, HIDDEN], BF16)
            nc.sync.dma_start(out=a2t[:], in_=inA2[:])
            b1t = cst.tile([128, 2 * R], BF16)
            nc.sync.dma_start(out=b1t[:], in_=inB1[:])
            b2t = cst.tile([128, 2 * W2W], BF16)
            nc.gpsimd.dma_start(out=b2t[:], in_=inB2[:])
            auxt = cst.tile([128, 6], F32)
            nc.gpsimd.dma_start(out=auxt[:], in_=aux[:])

            xt = a1t[:, 0:R]
            w1x = a1t[:, R:R + HIDDEN]
            w1h = a2t[:]
            hst = b1t[:]
            w2b = [b2t[:, br * NCH * DIM_X:(br + 1) * NCH * DIM_X] for br in range(2)]
            cmb = [b2t[:, 2 * NCH * DIM_X + br * NCH:2 * NCH * DIM_X + (br + 1) * NCH]
                   for br in range(2)]

            # both branches accumulate into the same banks (weights pre-scaled
            # by gs/(1-gs), so the sum IS the guidance-combined result)
            pv = psv.tile([128, R], F32)
            pd = psv.tile([1, R], F32)

            # hd = h_null - h: lets branch-null reuse branch-h's PSUM
            # (a_null = a_h + hd@W1h) instead of recomputing x@W1x
            hdt = cst.tile([128, R], BF16)
            nc.vector.tensor_tensor(hdt[:], hst[:, R:2 * R], hst[:, 0:R], op=ALU.subtract)

            # per-(chunk, branch) pieces: finer ACT/PSUM granularity keeps the
            # PE from stalling at chunk boundaries (rotating 1-bank a-tiles)
            for c in range(NCH):
                cs = bass.ts(c, 128)
                a = psa.tile([128, R], F32, tag="a")
                nc.tensor.matmul(a[:], w1x[:, cs], xt[:], start=True, stop=False)
                nc.tensor.matmul(a[:], w1h[:, cs], hst[:, 0:R], start=False, stop=True)
                for br in range(2):
                    first = c == 0 and br == 0
                    last = c == NCH - 1 and br == 1
                    if br == 1:
                        # after the br=0 tanh has read a, turn it into a_null
                        nc.tensor.matmul(a[:], w1h[:, cs], hdt[:], start=False,
                                         stop=True, skip_group_check=True)
                    u = actp.tile([128, R], BF16, tag="u")
                    nc.scalar.activation(u[:], a[:], AF.Tanh, bias=auxt[:, c:c + 1], scale=1.0)
                    u2 = actp.tile([128, R], BF16, tag="u2")
                    nc.vector.tensor_tensor(u2[:], u[:], u[:], op=ALU.mult)

                    nc.tensor.matmul(pv[:], w2b[br][:, cs], u[:], start=first, stop=last)
                    nc.tensor.matmul(pd[0:1, :], cmb[br][:, c:c + 1], u2[:], start=first, stop=last)

            # weights pre-scaled by gs/(1-gs)/-gs/-(1-gs): the PSUM sums ARE the
            # guidance-combined results; just add the bias terms. vout on ACT
            # and dout on DVE so the two PSUM->SBUF moves run in parallel.
            vout = outp.tile([128, R], F32)
            nc.scalar.activation(vout[:], pv[:], AF.Identity, bias=auxt[:, 4:5], scale=1.0)
            dout = outp.tile([1, R], F32)
            nc.vector.tensor_scalar(dout[:], pd[0:1, :], auxt[0:1, 5:6], None, op0=ALU.add)

            nc.sync.dma_start(out=VO[:], in_=vout[:])
            nc.scalar.dma_start(out=DO[:], in_=dout[:])
    nc.compile()
    return nc


def _get_nc():
    global _NC_CACHE
    if _NC_CACHE is None:
        _NC_CACHE = _build()
    return _NC_CACHE


def _prep_in_maps(state, h, h_null, t, guidance_scale, W1, b1, W2, b2):
    f32 = np.float32
    bf = ml_dtypes.bfloat16
    xTf = state[:, :DIM_X].T.astype(bf)                            # (128, B)
    hTf = h.T.astype(bf)
    hnTf = h_null.T.astype(bf)
    w1f = np.concatenate([W1[:DIM_X], W1[DIM_X:DIM_X + DIM_H]], axis=1).astype(bf)
    b1p = (b1.astype(f32) + t.astype(f32)[0] * W1[DIM_X + DIM_H].astype(f32))
    w2r = W2.astype(f32).reshape(NCH, 128, DIM_X).transpose(1, 0, 2).reshape(128, NCH * DIM_X)
    cvec = (W1[:DIM_X].astype(np.float64) * W2.astype(np.float64).T).sum(0)  # (512,)
    d0 = cvec.sum()
    cmatf = cvec.reshape(NCH, 128).T.astype(f32)                   # (128, NCH)
    gs = float(guidance_scale.astype(f32)[0])
    w2cf = np.concatenate([gs * w2r, (1.0 - gs) * w2r,
                           -gs * cmatf, -(1.0 - gs) * cmatf], axis=1).astype(bf)

    auxf = np.zeros((128, 6), f32)
    auxf[:, 0:4] = b1p.reshape(NCH, 128).T
    auxf[:, 4] = b2.astype(f32)
    auxf[:, 5] = d0

    w1xa = np.ascontiguousarray(w1f[:, :HIDDEN])
    w1ha = np.ascontiguousarray(w1f[:, HIDDEN:])
    in_maps = []
    for i in range(N_CORES):
        sl = slice(i * R, (i + 1) * R)
        in_maps.append({
            "inA1": np.ascontiguousarray(
                np.concatenate([xTf[:, sl], w1xa], axis=1)),
            "inA2": w1ha,
            "inB1": np.ascontiguousarray(
                np.concatenate([hTf[:, sl], hnTf[:, sl]], axis=1)),
            "inB2": w2cf,
            "aux": auxf,
        })
    return in_maps


def kernel(state, h, h_null, t, guidance_scale, W1, b1, W2, b2, _trace=False):
    nc = _get_nc()
    in_maps = _prep_in_maps(state, h, h_null, t, guidance_scale, W1, b1, W2, b2)
    res = run_bass_kernel_spmd(nc, in_maps, list(range(N_CORES)), trace=_trace)
    out = np.empty((B, DIM_X + 1), np.float32)
    for i in range(N_CORES):
        sl = slice(i * R, (i + 1) * R)
        out[sl, :DIM_X] = res.results[i]["VO"].T
        out[sl, DIM_X] = res.results[i]["DO"][0]
    if _trace:
        return out, res
    return out
